# revision 4
# baseline (speedup 1.0000x reference)
"""MoE decoder Trainium2 kernel (nn_MoEDecoder_67654324846797).

Strategy
--------
Data-parallel: the token dim (N=65536) is sharded across 8 NeuronCores
(8192 tokens each); all weights are replicated. No collectives.

v2 changes vs the 571us baseline (all PE-side, since the kernel is
PE-bound at ~90% busy):
  - x is pre-transposed on the HOST (numpy) so the device consumes
    feature-major xT [512, 8192] directly: kills 16 PE transposes +
    4 PSUM->SBUF copies per 512-token tile. The output is produced
    feature-major outT [256, 8192] and transposed back on the host:
    kills 8 more PE transposes per tile.
  - The gating MLP runs in fp8 (e4m3) with DoubleRow matmuls (2 K-rows
    per cycle): weights are pre-scaled and quantized on the host with
    fixed power-of-2 scales, activations are quantized on the ACT
    engine with the dequant scales folded into the (host pre-scaled)
    biases. Measured contribution to end-to-end error: ~0.6% (budget
    2e-2). Expert matmuls stay f32r: e4m3 quantization there costs
    4-7% end-to-end, far over the tolerance.
  - L3/gate path in bf16 (h2, gate probs, eW3, eb3): halves the gate
    broadcast DMA and gets the 2x DVE tensor_tensor mode for the gate
    multiply.
  - Elementwise rebalanced: h1 bias+relu on ACT, h2 bias+relu and gate
    mul on DVE, so both stay under the PE's ~29.5us/tile.

Per-tile PE budget: experts 65536 cyc + gating(fp8) ~3584 + Z-sum 512 +
gated-bias 1024 = ~70.7K cyc @2.4GHz = 29.4us; 16 tiles -> ~475us
predicted vs 571us baseline.
"""

import numpy as np
import ml_dtypes

import concourse.bass as bass
import concourse.tile as tile
from concourse import bacc, mybir

F32 = mybir.dt.float32
F32R = mybir.dt.float32r
BF16 = mybir.dt.bfloat16
FP8 = mybir.dt.float8e4

NP_FP8 = ml_dtypes.float8_e4m3  # TRN e4m3: max normal 240
NP_BF16 = ml_dtypes.bfloat16

N_TOKENS = 65536
N_CORES = 8
TOK_PER_CORE = N_TOKENS // N_CORES  # 8192
TILE = 512  # tokens per tile
N_TILES = TOK_PER_CORE // TILE  # 16
IN_CH = 512
HID = 256
OUT_CH = 256
E = 8

# fp8 gating scales (powers of two; dequant is exact).
# |x| <= ~5.5 -> xq8 max ~176; |gW1| <= 1/sqrt(512) -> *4096 max ~181;
# |gW2|,|gW3| <= 1/16 -> *2048 max 128; g1,g2 activations *32 stay < 224.
SX = 32.0
SW1 = 4096.0
SW2 = 2048.0
SW3 = 2048.0
SG1 = 32.0
SG2 = 32.0

RELU = mybir.ActivationFunctionType.Relu
EXP = mybir.ActivationFunctionType.Exp
COPY = mybir.ActivationFunctionType.Copy
DOUBLE_ROW = mybir.MatmulPerfMode.DoubleRow


def build_kernel(time_reps: int = 1) -> bass.Bass:
    """Build the per-core SPMD program. time_reps>1 wraps the main loop in a
    hardware repeat loop (same work each iteration) for timing."""
    nc = bacc.Bacc("TRN2", target_bir_lowering=False, debug=False,
                   num_devices=N_CORES)

    # x is host-pre-transposed to feature-major [512, 8192]
    x = nc.dram_tensor("x", [IN_CH, TOK_PER_CORE], F32R, kind="ExternalInput").ap()
    eW1 = nc.dram_tensor("eW1", [E, IN_CH, HID], F32R, kind="ExternalInput").ap()
    eb1 = nc.dram_tensor("eb1", [E, HID], F32, kind="ExternalInput").ap()
    eW2 = nc.dram_tensor("eW2", [E, HID, HID], F32R, kind="ExternalInput").ap()
    eb2 = nc.dram_tensor("eb2", [E, HID], F32, kind="ExternalInput").ap()
    eW3 = nc.dram_tensor("eW3", [E, HID, OUT_CH], BF16, kind="ExternalInput").ap()
    eb3 = nc.dram_tensor("eb3", [E, OUT_CH], BF16, kind="ExternalInput").ap()
    # gating weights are host-quantized fp8 (pre-scaled), biases pre-scaled f32
    gW1 = nc.dram_tensor("gW1", [IN_CH, HID], FP8, kind="ExternalInput").ap()
    gb1 = nc.dram_tensor("gb1", [HID], F32, kind="ExternalInput").ap()
    gW2 = nc.dram_tensor("gW2", [HID, HID], FP8, kind="ExternalInput").ap()
    gb2 = nc.dram_tensor("gb2", [HID], F32, kind="ExternalInput").ap()
    gW3 = nc.dram_tensor("gW3", [HID, E], FP8, kind="ExternalInput").ap()
    gb3 = nc.dram_tensor("gb3", [E], F32, kind="ExternalInput").ap()
    # output is produced feature-major [256, 8192]; host transposes back
    out = nc.dram_tensor("out", [OUT_CH, TOK_PER_CORE], F32, kind="ExternalOutput").ap()

    with tile.TileContext(nc) as tc:
        _body(nc, tc, x, eW1, eb1, eW2, eb2, eW3, eb3,
              gW1, gb1, gW2, gb2, gW3, gb3, out, time_reps)
    nc.compile()
    return nc


def _body(nc, tc, x, eW1, eb1, eW2, eb2, eW3, eb3,
          gW1, gb1, gW2, gb2, gW3, gb3, out, time_reps):
    from contextlib import ExitStack

    ctx = ExitStack()
    with ctx:
        wpool = ctx.enter_context(tc.tile_pool(name="wpool", bufs=1))
        io_pool = ctx.enter_context(tc.tile_pool(name="io", bufs=3))
        act_pool = ctx.enter_context(tc.tile_pool(name="act", bufs=2))
        small_pool = ctx.enter_context(tc.tile_pool(name="small", bufs=2))
        ps_mlp = ctx.enter_context(tc.tile_pool(name="ps_mlp", bufs=4, space="PSUM"))
        ps_out = ctx.enter_context(tc.tile_pool(name="ps_out", bufs=1, space="PSUM"))
        ps_g = ctx.enter_context(tc.tile_pool(name="ps_g", bufs=2, space="PSUM"))
        dram_pool = ctx.enter_context(tc.tile_pool(name="dram", bufs=3, space="DRAM"))

        # ---- prefetch x for the first tiles so the weight stream doesn't
        # delay the first gating/expert work ----
        x_r = x.rearrange("(kt kp) (t s) -> t kp kt s", kp=128, s=TILE)
        x_t = {}

        def load_x(t):
            xT = io_pool.tile([128, 4, TILE], F32R, name="xT")
            nc.sync.dma_start(xT, x_r[t])
            x_t[t] = xT

        if time_reps == 1:
            load_x(0)
            load_x(1)
            load_x(2)

        # ---- weight preload (feature-major, stationary layouts) ----
        g1w = wpool.tile([128, 4, HID], FP8, name="g1w")
        nc.sync.dma_start(g1w, gW1.rearrange("(kt kp) m -> kp kt m", kp=128))
        g2w = wpool.tile([128, 2, HID], FP8, name="g2w")
        nc.sync.dma_start(g2w, gW2.rearrange("(kt kp) m -> kp kt m", kp=128))
        # padded to 16 cols: DoubleRow requires 16-element-aligned pair stride
        g3w = wpool.tile([128, 2, 16], FP8, name="g3w")
        nc.sync.dma_start(g3w[:, :, 0:E], gW3.rearrange("(kt kp) m -> kp kt m", kp=128))
        g1b = wpool.tile([128, 2], F32, name="g1b")
        nc.sync.dma_start(g1b, gb1.rearrange("(mt mp) -> mp mt", mp=128))
        g2b = wpool.tile([128, 2], F32, name="g2b")
        nc.sync.dma_start(g2b, gb2.rearrange("(mt mp) -> mp mt", mp=128))
        g3b = wpool.tile([E, 1], F32, name="g3b")
        nc.sync.dma_start(g3b, gb3.rearrange("(e one) -> e one", one=1))
        b1e = wpool.tile([128, E, 2], F32, name="b1e")
        nc.sync.dma_start(b1e, eb1.rearrange("e (mt mp) -> mp e mt", mp=128))
        b2e = wpool.tile([128, E, 2], F32, name="b2e")
        nc.sync.dma_start(b2e, eb2.rearrange("e (mt mp) -> mp e mt", mp=128))
        b3e = wpool.tile([E, OUT_CH], BF16, name="b3e")  # lhsT for bias matmul
        nc.sync.dma_start(b3e, eb3)
        w1e = wpool.tile([128, E, 4, HID], F32R, name="w1e")
        w2e = wpool.tile([128, E, 2, HID], F32R, name="w2e")
        w3e = wpool.tile([128, E, 2, OUT_CH], BF16, name="w3e")
        eW1r = eW1.rearrange("e (kt kp) m -> e kp kt m", kp=128)
        eW2r = eW2.rearrange("e (kt kp) m -> e kp kt m", kp=128)
        eW3r = eW3.rearrange("e (kt kp) m -> e kp kt m", kp=128)
        rings = [nc.sync, nc.scalar, nc.gpsimd]
        for e in range(E):
            ring = rings[e % 3]
            ring.dma_start(w1e[:, e], eW1r[e])
            ring.dma_start(w2e[:, e], eW2r[e])
            ring.dma_start(w3e[:, e], eW3r[e])

        ones8 = wpool.tile([E, 1], F32, name="ones8")
        nc.vector.memset(ones8, 1.0)
        ones8r = wpool.tile([E, 1], F32R, name="ones8r")
        nc.vector.tensor_copy(ones8r, ones8)

        out_r = out.rearrange("(mt mp) (t s) -> t mp mt s", mp=128, s=TILE)

        # Pipelined 2-phase structure: phase A (load x, quantize for gating,
        # gating MLP, probability broadcast DMA chain) runs 2 tiles ahead of
        # phase B (experts) so the w_bc DRAM-bounce latency is hidden behind
        # B's PE work.
        xT_t, wbc_t, probT_t = {}, {}, {}

        def phase_a(t):
            if t not in x_t:
                load_x(t)
            xT = x_t.pop(t)

            # quantize x for the fp8 gating MLP
            xq8 = act_pool.tile([128, 4, TILE], FP8, name="xq8", bufs=2)
            nc.scalar.activation(xq8, xT, COPY, scale=SX)

            g1q8 = act_pool.tile([128, 2, TILE], FP8, name="g1q8", bufs=1)
            for mt in range(2):
                p_g = ps_mlp.tile([128, TILE], F32, name="p_g", tag="pmlp")
                for kp in (0, 2):
                    nc.tensor.matmul(p_g, g1w[:, kp:kp + 2, mt * 128:(mt + 1) * 128],
                                     xq8[:, kp:kp + 2, :], perf_mode=DOUBLE_ROW,
                                     start=(kp == 0), stop=(kp == 2))
                nc.scalar.activation(g1q8[:, mt, :], p_g, RELU,
                                     bias=g1b[:, mt:mt + 1], scale=SG1 / (SW1 * SX))
            g2q8 = act_pool.tile([128, 2, TILE], FP8, name="g2q8", bufs=1)
            for mt in range(2):
                p_g2 = ps_mlp.tile([128, TILE], F32, name="p_g2", tag="pmlp")
                nc.tensor.matmul(p_g2, g2w[:, 0:2, mt * 128:(mt + 1) * 128],
                                 g1q8[:, 0:2, :], perf_mode=DOUBLE_ROW,
                                 start=True, stop=True)
                nc.scalar.activation(g2q8[:, mt, :], p_g2, RELU,
                                     bias=g2b[:, mt:mt + 1], scale=SG2 / (SW2 * SG1))
            p_l = ps_g.tile([E, TILE], F32, name="p_l", tag="pg")
            nc.tensor.matmul(p_l, g3w[:, :, 0:E], g2q8, perf_mode=DOUBLE_ROW,
                             start=True, stop=True)
            expT = small_pool.tile([E, TILE], F32R, name="expT")
            nc.scalar.activation(expT, p_l, EXP, bias=g3b, scale=1.0 / (SW3 * SG2))

            # Z = sum_e exp_e; r = 1/Z; prob = exp * r (normalized gate probs)
            p_z = ps_g.tile([1, TILE], F32, name="p_z", tag="pg")
            nc.tensor.matmul(p_z, ones8r, expT, start=True, stop=True)
            r_sb = small_pool.tile([1, TILE], F32, name="r_sb")
            nc.vector.reciprocal(r_sb, p_z)
            r_dram = dram_pool.tile([1, TILE], F32, name="r_dram")
            nc.gpsimd.dma_start(r_dram, r_sb)
            rb8 = small_pool.tile([E, TILE], F32, name="rb8")
            nc.gpsimd.dma_start(rb8, r_dram[0, :].partition_broadcast(E))
            probT = small_pool.tile([E, TILE], BF16, name="probT", bufs=3)
            nc.vector.tensor_mul(probT, expT, rb8)
            prob_dram = dram_pool.tile([E, TILE], BF16, name="prob_dram")
            nc.gpsimd.dma_start(prob_dram, probT)
            w_bc = []
            for e in range(E):
                wbe = act_pool.tile([128, TILE], BF16, name=f"wbe{e}", tag="wbc",
                                    bufs=16)
                nc.gpsimd.dma_start(
                    wbe, prob_dram[e, :].partition_broadcast(128))
                w_bc.append(wbe)
            xT_t[t], wbc_t[t], probT_t[t] = xT, w_bc, probT

        def phase_b(t):
            xT, w_bc, probT = xT_t.pop(t), wbc_t.pop(t), probT_t.pop(t)
            p_o = [ps_out.tile([128, TILE], F32, name=f"p_o{mt}", tag=f"po{mt}")
                   for mt in range(2)]
            for e in range(E):
                h1T = act_pool.tile([128, 2, TILE], F32R, name="h1T", bufs=3)
                for mt in range(2):
                    p_h = ps_mlp.tile([128, TILE], F32, name="p_h", tag="pmlp")
                    for kt in range(4):
                        nc.tensor.matmul(p_h, w1e[:, e, kt, mt * 128:(mt + 1) * 128],
                                         xT[:, kt, :], start=(kt == 0), stop=(kt == 3))
                    nc.scalar.activation(h1T[:, mt, :], p_h, RELU,
                                         bias=b1e[:, e, mt:mt + 1])
                h2s = act_pool.tile([128, 2, TILE], BF16, name="h2s")
                for mt in range(2):
                    p_h2 = ps_mlp.tile([128, TILE], F32, name="p_h2", tag="pmlp")
                    for kt in range(2):
                        nc.tensor.matmul(p_h2, w2e[:, e, kt, mt * 128:(mt + 1) * 128],
                                         h1T[:, kt, :], start=(kt == 0), stop=(kt == 1))
                    h2T = act_pool.tile([128, TILE], BF16, name="h2T", bufs=3)
                    nc.vector.tensor_scalar(
                        h2T, p_h2, b2e[:, e, mt:mt + 1], 0.0,
                        mybir.AluOpType.add, mybir.AluOpType.max)
                    nc.vector.tensor_mul(h2s[:, mt, :], h2T, w_bc[e])
                for mt in range(2):
                    for kt in range(2):
                        nc.tensor.matmul(p_o[mt], w3e[:, e, kt, mt * 128:(mt + 1) * 128],
                                         h2s[:, kt, :],
                                         start=(e == 0 and kt == 0), stop=False,
                                         skip_group_check=True)

            # gated bias: p_o[mt] += eb3.T[mt-slice] @ probT
            for mt in range(2):
                nc.tensor.matmul(p_o[mt], b3e[:, mt * 128:(mt + 1) * 128], probT,
                                 start=False, stop=True, skip_group_check=True)

            outT = io_pool.tile([128, 2, TILE], F32, name="outT", bufs=2)
            for mt in range(2):
                nc.vector.tensor_copy(outT[:, mt, :], p_o[mt])
            nc.sync.dma_start(out_r[t], outT)

        def main_loop():
            if time_reps > 1:
                load_x(0)
                load_x(1)
            phase_a(0)
            phase_a(1)
            for t in range(N_TILES):
                if t + 2 < N_TILES:
                    phase_a(t + 2)
                phase_b(t)

        if time_reps > 1:
            with tc.For_i(0, time_reps, 1):
                main_loop()
        else:
            main_loop()


# ---------------------------------------------------------------------------
# PJRT runner (self-contained; mirrors concourse.bass2jax.run_bass_via_pjrt
# but keeps the jitted callable + device inputs for repeat timing)
# ---------------------------------------------------------------------------
class BassRunner:
    def __init__(self, nc: bass.Bass, n_cores: int = 8):
        import jax
        from jax.sharding import Mesh, PartitionSpec
        from jax.experimental.shard_map import shard_map
        from concourse.bass2jax import (
            _bass_exec_p, install_neuronx_cc_hook, partition_id_tensor,
        )

        install_neuronx_cc_hook()
        self.jax = jax
        self.nc = nc
        self.n_cores = n_cores
        partition_name = (
            nc.partition_id_tensor.name if nc.partition_id_tensor else None
        )

        in_names, out_names, out_avals, zero_shapes = [], [], [], []
        for alloc in nc.m.functions[0].allocations:
            if not isinstance(alloc, mybir.MemoryLocationSet):
                continue
            name = alloc.memorylocations[0].name
            if alloc.kind == "ExternalInput":
                if name != partition_name:
                    in_names.append(name)
            elif alloc.kind == "ExternalOutput":
                shape = tuple(alloc.tensor_shape)
                np_dt = mybir.dt.np(alloc.dtype)
                out_names.append(name)
                out_avals.append(jax.core.ShapedArray(shape, np_dt))
                zero_shapes.append((shape, np_dt))

        self.in_names, self.out_names = in_names, out_names
        self.out_avals, self.zero_shapes = out_avals, zero_shapes
        n_params, n_outs = len(in_names), len(out_names)
        bind_in_names = in_names + out_names
        if partition_name is not None:
            bind_in_names.append(partition_name)

        def _b(*args):
            operands = list(args)
            if partition_name is not None:
                operands.append(partition_id_tensor())
            return tuple(_bass_exec_p.bind(
                *operands,
                out_avals=tuple(out_avals),
                in_names=tuple(bind_in_names),
                out_names=tuple(out_names),
                lowering_input_output_aliases=(),
                sim_require_finite=True,
                sim_require_nnan=True,
                nc=nc,
            ))

        devices = jax.devices()[:n_cores]
        assert len(devices) == n_cores
        self.mesh = Mesh(np.asarray(devices), ("core",))
        self.pspec = PartitionSpec("core")
        in_specs = (self.pspec,) * (n_params + n_outs)
        out_specs = (self.pspec,) * n_outs
        self.sharded = jax.jit(
            shard_map(_b, mesh=self.mesh, in_specs=in_specs,
                      out_specs=out_specs, check_rep=False),
            keep_unused=True,
        )
        self._dev_in = None

    def put_inputs(self, in_maps):
        import jax
        concat = [
            np.concatenate([in_maps[c][n] for c in range(self.n_cores)], axis=0)
            for n in self.in_names
        ]
        zeros = [
            np.zeros((self.n_cores * s[0], *s[1:]), d) for s, d in self.zero_shapes
        ]
        sh = jax.sharding.NamedSharding(self.mesh, self.pspec)
        self._dev_in = [jax.device_put(a, sh) for a in concat + zeros]
        jax.block_until_ready(self._dev_in)

    def run(self):
        out = self.sharded(*self._dev_in)
        self.jax.block_until_ready(out)
        return out

    def results(self, out):
        res = []
        for c in range(self.n_cores):
            d = {}
            for i, name in enumerate(self.out_names):
                arr = np.asarray(out[i]).reshape(
                    self.n_cores, *self.out_avals[i].shape)
                d[name] = arr[c]
            res.append(d)
        return res

    def time_runs(self, iters=10, warmup=2):
        import time
        for _ in range(warmup):
            self.run()
        times = []
        for _ in range(iters):
            t0 = time.perf_counter()
            self.run()
            times.append(time.perf_counter() - t0)
        return min(times), sum(times) / len(times)


_cached = {}


def _get_runner(time_reps: int = 1) -> BassRunner:
    if time_reps not in _cached:
        nc = build_kernel(time_reps)
        _cached[time_reps] = BassRunner(nc, N_CORES)
    return _cached[time_reps]


def _in_maps(inputs: dict) -> list:
    f32 = lambda k: np.ascontiguousarray(np.asarray(inputs[k], dtype=np.float32))
    bf16 = lambda k: np.ascontiguousarray(np.asarray(inputs[k], dtype=np.float32)
                                          .astype(NP_BF16))

    def q8(k, s):
        v = np.asarray(inputs[k], dtype=np.float32) * s
        return np.ascontiguousarray(v.astype(NP_FP8))

    shared = {
        "eW1": f32("eW1"), "eb1": f32("eb1"),
        "eW2": f32("eW2"), "eb2": f32("eb2"),
        "eW3": bf16("eW3"), "eb3": bf16("eb3"),
        "gW1": q8("gW1", SW1),
        "gb1": np.ascontiguousarray(np.asarray(inputs["gb1"], np.float32) * SG1),
        "gW2": q8("gW2", SW2),
        "gb2": np.ascontiguousarray(np.asarray(inputs["gb2"], np.float32) * SG2),
        "gW3": q8("gW3", SW3),
        "gb3": f32("gb3"),
    }
    x_full = np.asarray(inputs["x"], dtype=np.float32)
    maps = []
    for c in range(N_CORES):
        m = dict(shared)
        m["x"] = np.ascontiguousarray(
            x_full[c * TOK_PER_CORE:(c + 1) * TOK_PER_CORE].T)
        maps.append(m)
    return maps


def kernel(**inputs) -> np.ndarray:
    runner = _get_runner(1)
    runner.put_inputs(_in_maps(inputs))
    res = runner.results(runner.run())
    return np.concatenate([r["out"].T for r in res], axis=0)


# revision 15
# speedup vs baseline: 1.1949x; 1.1949x over previous
"""MoE decoder Trainium2 kernel (nn_MoEDecoder_67654324846797).

Strategy
--------
Data-parallel: the token dim (N=65536) is sharded across 8 NeuronCores
(8192 tokens each); all weights are replicated. No collectives.

Key points vs the 571us baseline (PE-bound at ~90%):
  - x is pre-transposed (and bf16-cast) on the HOST so the device
    consumes feature-major xT [512, 8192] directly: no PE transposes.
    The output is produced feature-major outT [256, 8192] f32 and
    transposed back on the host.
  - The gating MLP runs in fp8 (e4m3) with DoubleRow matmuls (2x PE
    throughput): weights host-quantized with fixed power-of-2 scales,
    activations quantized on-chip with dequant scales folded into the
    (host pre-scaled) biases.
  - Gates are UNNORMALIZED exp(logit); the softmax 1/Z is applied at
    the output eviction (out = (sum_e exp_e*expert_e + exp@b3)/Z).
    This moves the Z -> reciprocal -> partition-broadcast chain off
    the PE critical path.
  - Expert path all-bf16 (same PE rate as f32r on TRN2, half the DMA
    and SBUF): measured end-to-end rel err ~7.4e-3 vs budget 2e-2.
  - 4-deep staged software pipeline: gating L1 for tile t+3, L2 for
    t+2, L3+exp+gate-broadcast for t+1, experts for t, all emitted so
    every in-order PE instruction's operands were produced >=1 tile
    earlier. Expert L3 matmuls are emitted two experts late so the
    DVE h2 bias+gate chain is never waited on.
  - Per-token gate/1/Z broadcasts across partitions go through a DRAM
    bounce on the SWDGE ring (engines can't read 0-stride partition
    APs); issued ~a tile ahead of use.
"""

import numpy as np
import ml_dtypes

import concourse.bass as bass
import concourse.tile as tile
from concourse import bacc, mybir

F32 = mybir.dt.float32
F32R = mybir.dt.float32r
BF16 = mybir.dt.bfloat16
FP8 = mybir.dt.float8e4

NP_FP8 = ml_dtypes.float8_e4m3  # TRN e4m3: max normal 240
NP_BF16 = ml_dtypes.bfloat16

N_TOKENS = 65536
N_CORES = 8
TOK_PER_CORE = N_TOKENS // N_CORES  # 8192
TILE = 512  # tokens per tile
N_TILES = TOK_PER_CORE // TILE  # 16
IN_CH = 512
HID = 256
OUT_CH = 256
E = 8

# fp8 gating scales (powers of two; dequant is exact).
# |x| <= ~5.5 -> xq8 max ~176; |gW1| <= 1/sqrt(512) -> *4096 max ~181;
# |gW2|,|gW3| <= 1/16 -> *2048 max 128; g1,g2 activations *32 stay < 224.
SX = 32.0
SW1 = 4096.0
SW2 = 2048.0
SW3 = 2048.0
SG1 = 32.0
SG2 = 32.0

RELU = mybir.ActivationFunctionType.Relu
EXP = mybir.ActivationFunctionType.Exp
DOUBLE_ROW = mybir.MatmulPerfMode.DoubleRow


def build_kernel(time_reps: int = 1) -> bass.Bass:
    """Build the per-core SPMD program. time_reps>1 wraps the main loop in a
    hardware repeat loop (same work each iteration) for timing."""
    nc = bacc.Bacc("TRN2", target_bir_lowering=False, debug=False,
                   num_devices=N_CORES)

    # x is host-pre-transposed to feature-major [512, 8192], bf16
    x = nc.dram_tensor("x", [IN_CH, TOK_PER_CORE], BF16, kind="ExternalInput").ap()
    eW1 = nc.dram_tensor("eW1", [E, IN_CH, HID], BF16, kind="ExternalInput").ap()
    eW2 = nc.dram_tensor("eW2", [E, HID, HID], BF16, kind="ExternalInput").ap()
    eW3 = nc.dram_tensor("eW3", [E, HID, OUT_CH], BF16, kind="ExternalInput").ap()
    eb3 = nc.dram_tensor("eb3", [E, OUT_CH], BF16, kind="ExternalInput").ap()
    # packed per-partition expert biases: eb{1,2}p[mp, e*2+mt] = eb{1,2}[e, mt*128+mp]
    eb1p = nc.dram_tensor("eb1p", [128, E * 2], F32, kind="ExternalInput").ap()
    eb2p = nc.dram_tensor("eb2p", [128, E * 2], F32, kind="ExternalInput").ap()
    # gating weights host-quantized fp8 (pre-scaled); packed biases
    # gbp[:, 0:2]=gb1*SG1, [:, 2:4]=gb2*SG2, [0:8, 4]=gb3
    gW1 = nc.dram_tensor("gW1", [IN_CH, HID], FP8, kind="ExternalInput").ap()
    gW2 = nc.dram_tensor("gW2", [HID, HID], FP8, kind="ExternalInput").ap()
    gW3 = nc.dram_tensor("gW3", [HID, E], FP8, kind="ExternalInput").ap()
    gbp = nc.dram_tensor("gbp", [128, 6], F32, kind="ExternalInput").ap()
    # output is produced feature-major [256, 8192]; host transposes back
    out = nc.dram_tensor("out", [OUT_CH, TOK_PER_CORE], F32, kind="ExternalOutput").ap()

    with tile.TileContext(nc) as tc:
        _body(nc, tc, x, eW1, eW2, eW3, eb3, eb1p, eb2p,
              gW1, gW2, gW3, gbp, out, time_reps)
    nc.compile()
    return nc


def _body(nc, tc, x, eW1, eW2, eW3, eb3, eb1p, eb2p,
          gW1, gW2, gW3, gbp, out, time_reps):
    from contextlib import ExitStack

    ctx = ExitStack()
    with ctx:
        wpool = ctx.enter_context(tc.tile_pool(name="wpool", bufs=1))
        io_pool = ctx.enter_context(tc.tile_pool(name="io", bufs=3))
        act_pool = ctx.enter_context(tc.tile_pool(name="act", bufs=2))
        small_pool = ctx.enter_context(tc.tile_pool(name="small", bufs=2))
        ps_mlp = ctx.enter_context(tc.tile_pool(name="ps_mlp", bufs=4, space="PSUM"))
        ps_out = ctx.enter_context(tc.tile_pool(name="ps_out", bufs=1, space="PSUM"))
        ps_g = ctx.enter_context(tc.tile_pool(name="ps_g", bufs=2, space="PSUM"))
        dram_pool = ctx.enter_context(tc.tile_pool(name="dram", bufs=3, space="DRAM"))

        x_r = x.rearrange("(kt kp) (t s) -> t kp kt s", kp=128, s=TILE)
        x_t = {}

        def load_x(t):
            # alive from the t+4 prefetch until tile t's experts: 5 concurrent
            xT = io_pool.tile([128, 4, TILE], BF16, name="xT", bufs=5)
            nc.sync.dma_start(xT, x_r[t])
            x_t[t] = xT

        # ---- weight preload ----
        # Startup critical path is x(0) -> xq8 -> gating chain -> first
        # experts, so the sync ring goes x(0), gating weights, x(1), then
        # expert weights interleaved with the remaining x prefetch tiles.
        # Packed biases (single contiguous run per partition) ride the
        # scalar ring.
        if time_reps == 1:
            load_x(0)
        g1w = wpool.tile([128, 4, HID], FP8, name="g1w")
        nc.sync.dma_start(g1w, gW1.rearrange("(kt kp) m -> kp kt m", kp=128))
        g2w = wpool.tile([128, 2, HID], FP8, name="g2w")
        nc.sync.dma_start(g2w, gW2.rearrange("(kt kp) m -> kp kt m", kp=128))
        # padded to 16 cols: DoubleRow requires 16-element-aligned pair stride
        g3w = wpool.tile([128, 2, 16], FP8, name="g3w")
        nc.sync.dma_start(g3w[:, :, 0:E], gW3.rearrange("(kt kp) m -> kp kt m", kp=128))
        gbt = wpool.tile([128, 6], F32, name="gbt")
        nc.scalar.dma_start(gbt, gbp)
        b1e = wpool.tile([128, E, 2], F32, name="b1e")
        nc.scalar.dma_start(b1e, eb1p.rearrange("p (e mt) -> p e mt", mt=2))
        b2e = wpool.tile([128, E, 2], F32, name="b2e")
        nc.scalar.dma_start(b2e, eb2p.rearrange("p (e mt) -> p e mt", mt=2))
        b3e = wpool.tile([E, OUT_CH], BF16, name="b3e")  # lhsT for bias matmul
        nc.scalar.dma_start(b3e, eb3)
        if time_reps == 1:
            load_x(1)

        w1e = wpool.tile([128, E, 4, HID], BF16, name="w1e")
        w2e = wpool.tile([128, E, 2, HID], BF16, name="w2e")
        w3e = wpool.tile([128, E, 2, OUT_CH], BF16, name="w3e")
        eW1r = eW1.rearrange("e (kt kp) m -> e kp kt m", kp=128)
        eW2r = eW2.rearrange("e (kt kp) m -> e kp kt m", kp=128)
        eW3r = eW3.rearrange("e (kt kp) m -> e kp kt m", kp=128)
        rings = [nc.sync, nc.scalar, nc.gpsimd]
        for e in range(E):
            ring = rings[e % 3]
            ring.dma_start(w1e[:, e], eW1r[e])
            ring.dma_start(w2e[:, e], eW2r[e])
            ring.dma_start(w3e[:, e], eW3r[e])
            if time_reps == 1 and e in (0, 2):
                load_x(2 + e // 2)

        ones8 = wpool.tile([E, 1], F32, name="ones8")
        nc.vector.memset(ones8, 1.0)
        ones8b = wpool.tile([E, 1], BF16, name="ones8b")
        nc.vector.tensor_copy(ones8b, ones8)

        out_r = out.rearrange("(mt mp) (t s) -> t mp mt s", mp=128, s=TILE)

        xq8_t, g1q8_t, g2q8_t, expT_t, wbc_t, zrb_t = {}, {}, {}, {}, {}, {}

        def stage_x(t):
            if t not in x_t:
                load_x(t)
            # quantize on DVE (tail of its queue; bf16 in -> 4x mode):
            # keeps the ACT queue free for the expert h1 evictions
            xq8 = act_pool.tile([128, 4, TILE], FP8, name="xq8", bufs=4)
            nc.vector.tensor_scalar(xq8, x_t[t], SX, None, mybir.AluOpType.mult)
            xq8_t[t] = xq8

        def stage_g1(t):
            xq8 = xq8_t.pop(t)
            g1q8 = act_pool.tile([128, 2, TILE], FP8, name="g1q8", bufs=2)
            for mt in range(2):
                p_g = ps_mlp.tile([128, TILE], F32, name="p_g", tag="pmlp")
                for kp in (0, 2):
                    nc.tensor.matmul(p_g, g1w[:, kp:kp + 2, mt * 128:(mt + 1) * 128],
                                     xq8[:, kp:kp + 2, :], perf_mode=DOUBLE_ROW,
                                     start=(kp == 0), stop=(kp == 2))
                nc.scalar.activation(g1q8[:, mt, :], p_g, RELU,
                                     bias=gbt[:, mt:mt + 1], scale=SG1 / (SW1 * SX))
            g1q8_t[t] = g1q8

        def stage_g2(t):
            g1q8 = g1q8_t.pop(t)
            g2q8 = act_pool.tile([128, 2, TILE], FP8, name="g2q8", bufs=2)
            for mt in range(2):
                p_g2 = ps_mlp.tile([128, TILE], F32, name="p_g2", tag="pmlp")
                nc.tensor.matmul(p_g2, g2w[:, 0:2, mt * 128:(mt + 1) * 128],
                                 g1q8[:, 0:2, :], perf_mode=DOUBLE_ROW,
                                 start=True, stop=True)
                nc.scalar.activation(g2q8[:, mt, :], p_g2, RELU,
                                     bias=gbt[:, 2 + mt:3 + mt], scale=SG2 / (SW2 * SG1))
            g2q8_t[t] = g2q8

        def stage_g3(t):
            g2q8 = g2q8_t.pop(t)
            p_l = ps_g.tile([E, TILE], F32, name="p_l", tag="pg")
            nc.tensor.matmul(p_l, g3w[:, :, 0:E], g2q8, perf_mode=DOUBLE_ROW,
                             start=True, stop=True)
            expT = small_pool.tile([E, TILE], BF16, name="expT", bufs=3)
            nc.scalar.activation(expT, p_l, EXP, bias=gbt[0:E, 4:5],
                                 scale=1.0 / (SW3 * SG2))
            exp_dram = dram_pool.tile([E, TILE], BF16, name="exp_dram")
            nc.gpsimd.dma_start(exp_dram, expT)
            w_bc = []
            for e in range(E):
                wbe = act_pool.tile([128, TILE], BF16, name=f"wbe{e}", tag="wbc",
                                    bufs=16)
                nc.gpsimd.dma_start(
                    wbe, exp_dram[e, :].partition_broadcast(128))
                w_bc.append(wbe)
            expT_t[t], wbc_t[t] = expT, w_bc

        def stage_z(t):
            # Z = sum_e exp_e; zr = 1/Z broadcast to all partitions
            p_z = ps_g.tile([1, TILE], F32, name="p_z", tag="pg")
            nc.tensor.matmul(p_z, ones8b, expT_t[t], start=True, stop=True)
            r_sb = small_pool.tile([1, TILE], F32, name="r_sb")
            nc.vector.reciprocal(r_sb, p_z)
            r_dram = dram_pool.tile([1, TILE], F32, name="r_dram")
            nc.gpsimd.dma_start(r_dram, r_sb)
            zrb = act_pool.tile([128, TILE], F32, name="zrb", tag="zrb", bufs=3)
            nc.gpsimd.dma_start(zrb, r_dram[0, :].partition_broadcast(128))
            zrb_t[t] = zrb

        def stage_experts(t):
            """Experts e0..e7 L1/L2 with each expert's L3 delayed TWO experts
            (absorbs the DVE h2T/h2s latency incl. its queue backlog).
            Returns a closure that emits the tail: L3(e6), L3(e7), gated
            bias, 1/Z output scaling, out DMA."""
            xT, w_bc, expT = x_t.pop(t), wbc_t.pop(t), expT_t.pop(t)
            p_o = [ps_out.tile([128, TILE], F32, name=f"p_o{mt}", tag=f"po{mt}")
                   for mt in range(2)]
            h2s_e = {}

            def emit_l3(e):
                h2s = h2s_e.pop(e)
                for mt in range(2):
                    for kt in range(2):
                        nc.tensor.matmul(p_o[mt], w3e[:, e, kt, mt * 128:(mt + 1) * 128],
                                         h2s[:, kt, :],
                                         start=(e == 0 and kt == 0), stop=False,
                                         skip_group_check=True)

            for e in range(E):
                h1T = act_pool.tile([128, 2, TILE], BF16, name="h1T", bufs=3)
                for mt in range(2):
                    p_h = ps_mlp.tile([128, TILE], F32, name="p_h", tag="pmlp")
                    for kt in range(4):
                        nc.tensor.matmul(p_h, w1e[:, e, kt, mt * 128:(mt + 1) * 128],
                                         xT[:, kt, :], start=(kt == 0), stop=(kt == 3))
                    nc.scalar.activation(h1T[:, mt, :], p_h, RELU,
                                         bias=b1e[:, e, mt:mt + 1])
                h2s = act_pool.tile([128, 2, TILE], BF16, name="h2s", bufs=4)
                for mt in range(2):
                    p_h2 = ps_mlp.tile([128, TILE], F32, name="p_h2", tag="pmlp")
                    for kt in range(2):
                        nc.tensor.matmul(p_h2, w2e[:, e, kt, mt * 128:(mt + 1) * 128],
                                         h1T[:, kt, :], start=(kt == 0), stop=(kt == 1))
                    h2T = act_pool.tile([128, TILE], BF16, name="h2T", bufs=3)
                    nc.vector.tensor_scalar(
                        h2T, p_h2, b2e[:, e, mt:mt + 1], 0.0,
                        mybir.AluOpType.add, mybir.AluOpType.max)
                    nc.vector.tensor_mul(h2s[:, mt, :], h2T, w_bc[e])
                h2s_e[e] = h2s
                if e > 1:
                    emit_l3(e - 2)

            def finish():
                emit_l3(E - 2)
                emit_l3(E - 1)
                # gated bias: p_o[mt] += eb3.T[mt-slice] @ expT (unnormalized)
                for mt in range(2):
                    nc.tensor.matmul(p_o[mt], b3e[:, mt * 128:(mt + 1) * 128], expT,
                                     start=False, stop=True, skip_group_check=True)
                zrb = zrb_t.pop(t)
                outT = io_pool.tile([128, 2, TILE], F32, name="outT", bufs=2)
                for mt in range(2):
                    nc.vector.tensor_mul(outT[:, mt, :], p_o[mt], zrb)
                nc.sync.dma_start(out_r[t], outT)

            return finish

        def main_loop():
            if time_reps > 1:
                for k in range(4):
                    load_x(k)
            for k in range(4):
                stage_x(k)
            stage_g1(0)
            stage_g1(1)
            stage_g1(2)
            stage_g2(0)
            stage_g2(1)
            stage_g3(0)
            stage_z(0)
            for t in range(N_TILES):
                # experts first: the ACT queue leads with h1 evictions, so
                # expert 0's L2 is never stuck behind gating evictions
                finish = stage_experts(t)
                # gating for future tiles + Z(t+1): PE work whose operands
                # are all >=1 tile old; also absorbs the last experts' DVE
                # h2s latency before finish() emits their L3
                if t + 1 < N_TILES:
                    stage_g3(t + 1)
                if t + 2 < N_TILES:
                    stage_g2(t + 2)
                if t + 3 < N_TILES:
                    stage_g1(t + 3)
                if t + 1 < N_TILES:
                    stage_z(t + 1)
                finish()
                if t + 4 < N_TILES:
                    stage_x(t + 4)

        if time_reps > 1:
            with tc.For_i(0, time_reps, 1):
                main_loop()
        else:
            main_loop()


# ---------------------------------------------------------------------------
# PJRT runner (self-contained; mirrors concourse.bass2jax.run_bass_via_pjrt
# but keeps the jitted callable + device inputs for repeat timing)
# ---------------------------------------------------------------------------
class BassRunner:
    def __init__(self, nc: bass.Bass, n_cores: int = 8):
        import jax
        from jax.sharding import Mesh, PartitionSpec
        from jax.experimental.shard_map import shard_map
        from concourse.bass2jax import (
            _bass_exec_p, install_neuronx_cc_hook, partition_id_tensor,
        )

        install_neuronx_cc_hook()
        self.jax = jax
        self.nc = nc
        self.n_cores = n_cores
        partition_name = (
            nc.partition_id_tensor.name if nc.partition_id_tensor else None
        )

        in_names, out_names, out_avals, zero_shapes = [], [], [], []
        for alloc in nc.m.functions[0].allocations:
            if not isinstance(alloc, mybir.MemoryLocationSet):
                continue
            name = alloc.memorylocations[0].name
            if alloc.kind == "ExternalInput":
                if name != partition_name:
                    in_names.append(name)
            elif alloc.kind == "ExternalOutput":
                shape = tuple(alloc.tensor_shape)
                np_dt = mybir.dt.np(alloc.dtype)
                out_names.append(name)
                out_avals.append(jax.core.ShapedArray(shape, np_dt))
                zero_shapes.append((shape, np_dt))

        self.in_names, self.out_names = in_names, out_names
        self.out_avals, self.zero_shapes = out_avals, zero_shapes
        n_params, n_outs = len(in_names), len(out_names)
        bind_in_names = in_names + out_names
        if partition_name is not None:
            bind_in_names.append(partition_name)

        def _b(*args):
            operands = list(args)
            if partition_name is not None:
                operands.append(partition_id_tensor())
            return tuple(_bass_exec_p.bind(
                *operands,
                out_avals=tuple(out_avals),
                in_names=tuple(bind_in_names),
                out_names=tuple(out_names),
                lowering_input_output_aliases=(),
                sim_require_finite=True,
                sim_require_nnan=True,
                nc=nc,
            ))

        devices = jax.devices()[:n_cores]
        assert len(devices) == n_cores
        self.mesh = Mesh(np.asarray(devices), ("core",))
        self.pspec = PartitionSpec("core")
        in_specs = (self.pspec,) * (n_params + n_outs)
        out_specs = (self.pspec,) * n_outs
        self.sharded = jax.jit(
            shard_map(_b, mesh=self.mesh, in_specs=in_specs,
                      out_specs=out_specs, check_rep=False),
            keep_unused=True,
        )
        self._dev_in = None

    def put_inputs(self, in_maps):
        import jax
        concat = [
            np.concatenate([in_maps[c][n] for c in range(self.n_cores)], axis=0)
            for n in self.in_names
        ]
        zeros = [
            np.zeros((self.n_cores * s[0], *s[1:]), d) for s, d in self.zero_shapes
        ]
        sh = jax.sharding.NamedSharding(self.mesh, self.pspec)
        self._dev_in = [jax.device_put(a, sh) for a in concat + zeros]
        jax.block_until_ready(self._dev_in)

    def run(self):
        out = self.sharded(*self._dev_in)
        self.jax.block_until_ready(out)
        return out

    def results(self, out):
        res = []
        for c in range(self.n_cores):
            d = {}
            for i, name in enumerate(self.out_names):
                arr = np.asarray(out[i]).reshape(
                    self.n_cores, *self.out_avals[i].shape)
                d[name] = arr[c]
            res.append(d)
        return res

    def time_runs(self, iters=10, warmup=2):
        import time
        for _ in range(warmup):
            self.run()
        times = []
        for _ in range(iters):
            t0 = time.perf_counter()
            self.run()
            times.append(time.perf_counter() - t0)
        return min(times), sum(times) / len(times)


_cached = {}


def _get_runner(time_reps: int = 1) -> BassRunner:
    if time_reps not in _cached:
        nc = build_kernel(time_reps)
        _cached[time_reps] = BassRunner(nc, N_CORES)
    return _cached[time_reps]


def _in_maps(inputs: dict) -> list:
    f32 = lambda k: np.asarray(inputs[k], dtype=np.float32)
    bf16 = lambda k: np.ascontiguousarray(f32(k).astype(NP_BF16))

    def q8(k, s):
        return np.ascontiguousarray((f32(k) * s).astype(NP_FP8))

    def pack_eb(k):
        # [E, 256] -> [128, E*2]: p[mp, e*2+mt] = b[e, mt*128+mp]
        b = f32(k).reshape(E, 2, 128).transpose(2, 0, 1).reshape(128, E * 2)
        return np.ascontiguousarray(b)

    gbp = np.zeros((128, 6), np.float32)
    gbp[:, 0:2] = (f32("gb1") * SG1).reshape(2, 128).T
    gbp[:, 2:4] = (f32("gb2") * SG2).reshape(2, 128).T
    gbp[0:E, 4] = f32("gb3")

    shared = {
        "eW1": bf16("eW1"), "eW2": bf16("eW2"),
        "eW3": bf16("eW3"), "eb3": bf16("eb3"),
        "eb1p": pack_eb("eb1"), "eb2p": pack_eb("eb2"),
        "gW1": q8("gW1", SW1), "gW2": q8("gW2", SW2), "gW3": q8("gW3", SW3),
        "gbp": gbp,
    }
    x_full = np.asarray(inputs["x"], dtype=np.float32)
    maps = []
    for c in range(N_CORES):
        m = dict(shared)
        m["x"] = np.ascontiguousarray(
            x_full[c * TOK_PER_CORE:(c + 1) * TOK_PER_CORE].T.astype(NP_BF16))
        maps.append(m)
    return maps


def kernel(**inputs) -> np.ndarray:
    runner = _get_runner(1)
    runner.put_inputs(_in_maps(inputs))
    res = runner.results(runner.run())
    return np.concatenate([r["out"].T for r in res], axis=0)
